# revision 41
# baseline (speedup 1.0000x reference)
"""Multi-head causal self-attention (B=2, T=4096, C=512, H=8) on 8 trn2 cores.

Sharding: 16 (batch, head) pairs -> 2 heads per core. Core c handles batch
c//4, heads {2*(c%4), 2*(c%4)+1}. Each core computes its heads' Q/K/V
projections from the (host-pre-transposed) activations, runs causal flash
attention, and applies its row-slice of the output projection; the host sums
the 4 partial outputs per batch.

Attention layout: scores are computed transposed ([tk, tq]) in fp32r into
PSUM; softmax weights are bf16. PV runs in [tq, d] layout (stationary =
attention tile window, moving = V[k,d]+ones), which packs the full 128
output partitions per pass and makes the softmax row-sum a per-partition
scalar: normalization is one reciprocal + broadcast-multiply. Causal
triangle masks are 0/1 multiplies on the bf16 weights, run post-exp on the
otherwise-idle Pool engine (PV trails by 2 pairs, so they are off the
critical path). The normalized output is PE-transposed back to [d, tq]
(bf16) for the output projection.

The ACT engine's exp over every causal score element (~17.6M elems/core at
128 lanes) is the throughput floor, so 1/3 of the non-diagonal score pairs
compute softmax weights on DVE instead, via a dual Schraudolph bitcast exp:
i_k = round(A*s + B_k) as int16 (B2 = B1+64), exp(s) ~= bf16_bits(i1) +
2^-0.5 * bf16_bits(i2). The two terms are combined by the PE itself (PV
runs twice, the second pass against a 2^-0.5-prescaled V copy), so the
offload costs DVE only one PSUM read (i1) plus one cheap int16 add. Max
weight error ~1%, which vanishes below bf16 noise after softmax
normalization (verified end-to-end).
"""

import numpy as np

import concourse.bass as bass
import concourse.mybir as mybir
import concourse.tile as tile
from concourse import bacc
from concourse.bass_utils import run_bass_kernel_spmd

B, T, C, H, D = 2, 4096, 512, 8, 64
NCORES = 8
SCALE = 1.0 / np.sqrt(D)

F32 = mybir.dt.float32
F32R = mybir.dt.float32r
BF16 = mybir.dt.bfloat16
I16 = mybir.dt.int16

TRACE = False
LAST_RESULT = None
# dual-Schraudolph exp offload (DVE/Pool) for non-diagonal score pairs:
# exp(x) ~= bf16_bits(round(A*x+B0-128)) + 2^-0.5 * bf16_bits(round(A*x+B0-64))
# (+-1.0% per weight, self-normalizing; end-to-end impact is below bf16 noise)
SCH_A = 128.0 / np.log(2.0)
SCH_B0 = 16249.125
SCH_B1 = SCH_B0 - 128.0
SCH_B2 = SCH_B0 - 64.0
SCH_C2 = 2.0 ** -0.5
SCH_MOD = 3
DEBUG = False  # adds intermediate dumps (dbg_*) for core-0 verification
DBG_GHP = (2, 0, 3)  # g, h, pair for the at_s dump

_NC = None


def _toff(d):
    """Column offset below which a diagonal block's scores are entirely
    invalid *and* skippable while keeping matmul N >= 256 (fp32r full rate)."""
    if d <= 0:
        return 0
    return 128 if d == 1 else 256


def _build(bias_free=False):
    nc = bacc.Bacc()

    xt = nc.declare_dram_parameter("xt", [4, 128, T], F32R, isOutput=False)
    wq = nc.declare_dram_parameter("wq", [4, 128, 128], F32R, isOutput=False)
    wk = nc.declare_dram_parameter("wk", [4, 128, 128], F32R, isOutput=False)
    wvt = nc.declare_dram_parameter("wvt", [4, 128, 128], F32R, isOutput=False)
    wout = nc.declare_dram_parameter("wout", [128, 4, 128], BF16, isOutput=False)
    # packed small constants: qb|kb|vbp|bout4|tri|ident16
    sblob = nc.declare_dram_parameter("sblob", [128, 135], F32, isOutput=False)
    out_t = nc.declare_dram_parameter("out_t", [C, T], F32, isOutput=True)
    if DEBUG:
        dbg_q = nc.declare_dram_parameter("dbg_q", [128, T], F32, isOutput=True)
        dbg_k = nc.declare_dram_parameter("dbg_k", [128, T], F32, isOutput=True)
        dbg_v = nc.declare_dram_parameter("dbg_v", [128, 32, 2, 65], F32,
                                          isOutput=True)
        dbg_at = nc.declare_dram_parameter("dbg_at", [128, 1024], F32,
                                           isOutput=True)
        dbg_o = nc.declare_dram_parameter("dbg_o", [128, 4, 65], F32,
                                          isOutput=True)
        dbg_on = nc.declare_dram_parameter("dbg_on", [128, 512], F32,
                                           isOutput=True)

    with tile.TileContext(nc) as tc:
        with (
            tc.tile_pool(name="w", bufs=1) as w,
            tc.tile_pool(name="sb", bufs=4) as sb,
            tc.tile_pool(name="sbA", bufs=11) as sbA,
            tc.tile_pool(name="sbI", bufs=9) as sbI,
            tc.tile_pool(name="psA", bufs=2, space="PSUM") as psA,
            tc.tile_pool(name="psO", bufs=2, space="PSUM") as psO,
            tc.tile_pool(name="psX", bufs=2, space="PSUM") as psX,
        ):
            # ---- weights / constants ----
            wq_s = w.tile([128, 4, 128], F32R)
            wk_s = w.tile([128, 4, 128], F32R)
            wvt_s = w.tile([128, 4, 128], F32R)
            wout_s = w.tile([128, 4, 128], BF16)
            sblob_s = w.tile([128, 135], F32)
            qb_s = sblob_s[:, 0:1]
            kb_s = sblob_s[:, 1:2]
            vbp_s = sblob_s[:, 2:3]
            bout_s = sblob_s[:, 3:7]
            ident_s = sblob_s[:, 7:71].bitcast(BF16)   # [128,128] bf16
            tri01_s = sblob_s[:, 71:135].bitcast(BF16)  # causal 0/1, bf16

            xt_s = w.tile([128, 4, T], F32R)
            qt_s = w.tile([128, T], F32R)  # partitions: [h0 q-dims | h1 q-dims]
            kt_s = w.tile([128, T], F32R)
            vt_s = w.tile([128, T], BF16)  # V^T stream: partitions [h0 d|h1 d]
            # per k-tile: [2 heads, 64 d + 1 ones]
            v16_s = w.tile([128, 32, 2, 65], BF16)
            v16c_s = w.tile([128, 32, 2, 65], BF16)  # 2^-0.5 * v16 (dual-schr)
            c2_s = w.tile([128, 1], BF16)


            def _proj_half(g, ws, dst, scale, bias, half, state, dt):
                sl = bass.ts(g, 512)
                if half == 0:
                    pproj = psX.tile([128, 512], F32, tag="x")
                    state["ps"] = pproj
                ps = state["ps"]
                for ch in (0, 1) if half == 0 else (2, 3):
                    nc.tensor.matmul(
                        ps, ws[:, ch, :], xt_s[:, ch, sl],
                        start=(ch == 0), stop=(ch == 3),
                    )
                if half == 1:
                    nc.vector.tensor_scalar(
                        dst[:, sl], ps, scale, bias,
                        mybir.AluOpType.mult, mybir.AluOpType.add,
                    )
                    state.pop("ps")

            def proj_q(g, half=None, state={}):
                for hf in (0, 1) if half is None else (half,):
                    _proj_half(g, wq_s, qt_s, SCALE, qb_s, hf, state, F32R)

            def proj_k(g, half=None, state={}):
                for hf in (0, 1) if half is None else (half,):
                    _proj_half(g, wk_s, kt_s, 1.0, kb_s, hf, state, F32R)

            def proj_vt(g, half=None, state={}):
                for hf in (0, 1) if half is None else (half,):
                    _proj_half(g, wvt_s, vt_s, 1.0, vbp_s, hf, state, BF16)

            def trans_v(g, t4):
                tt = g * 4 + t4
                pt = psX.tile([128, 512], F32, tag="x")
                ptb = pt.bitcast(BF16)
                nc.tensor.transpose(
                    ptb[:, 0:128], vt_s[:, bass.ts(tt, 128)], ident_s,
                )
                nc.vector.tensor_copy(
                    v16_s[:, tt, :, 0:64],
                    ptb[:, 0:128].rearrange("p (a b) -> p a b", a=2),
                )
                nc.gpsimd.tensor_tensor(
                    v16c_s[:, tt, :, :].rearrange("p a b -> p (a b)"),
                    v16_s[:, tt, :, :].rearrange("p a b -> p (a b)"),
                    c2_s.broadcast_to([128, 130]),
                    mybir.AluOpType.mult,
                )

            def proj(g, skip_dma=False):
                """QKV projection for column group g, emitted inline."""
                if not skip_dma:
                    sl = bass.ts(g, 512)
                    for ch in range(4):
                        nc.sync.dma_start(out=xt_s[:, ch, sl], in_=xt[ch][:, sl])
                proj_q(g)
                proj_k(g)
                proj_vt(g)
                for t4 in range(4):
                    trans_v(g, t4)

            def queue_proj(g):
                """Queue proj(g) pieces for drip-feeding under attention."""
                sl = bass.ts(g, 512)
                for ch in range(4):
                    nc.sync.dma_start(out=xt_s[:, ch, sl], in_=xt[ch][:, sl])
                for late, fn in ((0, proj_q), (1, proj_k), (1, proj_vt)):
                    st = {}
                    for hf in (0, 1):
                        proj_pending.append(
                            (g, late,
                             lambda g=g, fn=fn, hf=hf, st=st: fn(g, hf, st)))
                for t4 in range(4):
                    proj_pending.append(
                        (g, 1, lambda g=g, t4=t4: trans_v(g, t4)))

            def trans_o(g, qt, onorm16, onT):
                pt = psX.tile([128, 512], F32, tag="x")
                ptb = pt.bitcast(BF16)
                nc.tensor.transpose(
                    ptb[:, 0:128],
                    onorm16[:, qt].rearrange("p a b -> p (a b)"), ident_s,
                )
                nc.vector.tensor_copy(
                    onT[:, bass.ts(qt, 128)], ptb[:, 0:128])

            def outproj_m(g, onT, m, tail=False):
                """One column-chunk of the output projection for q-chunk g
                (deferred so it fills PE gaps under later attention)."""
                if tail:
                    op_full = psA.tile([128, 1024], F32, tag="bigA")
                    op_ps = op_full[:, 0:512]
                else:
                    op_ps = psX.tile([128, 512], F32, tag="x")
                nc.tensor.matmul(
                    op_ps, wout_s[:, m, :], onT,
                    start=True, stop=True,
                )
                oc_s = sb.tile([128, 512], F32, tag="outc")
                nc.vector.tensor_scalar(
                    oc_s, op_ps, 1.0, bout_s[:, m:m + 1],
                    mybir.AluOpType.mult, mybir.AluOpType.add,
                )
                nc.sync.dma_start(
                    out=out_t[bass.ts(m, 128), bass.ts(g, 512)], in_=oc_s
                )

            pv_pending = []
            deferred = []
            proj_pending = []

            def flush_pv(all=False):
                # keep up to 2 pending pv closures so PV matmuls only enter
                # the PE queue after their exp has certainly completed
                while pv_pending and (all or len(pv_pending) > 6):
                    pv_pending.pop(0)()

            def attn_g(g, onorm16):
                """Both heads' causal attention over q-chunk g, pair chains
                interleaved so the psA/at rotation always has an independent
                chain to run while the other waits on exp/conversions."""
                while proj_pending and (
                    proj_pending[0][0] < g
                    or (proj_pending[0][0] == g and proj_pending[0][1] == 0)
                ):
                    proj_pending.pop(0)[2]()
                o_ps = {h: psO.tile([128, 4, 128], F32, tag="o",
                                    name=f"o_ps{h}")
                        for h in (0, 1)}
                npairs = 2 * g + 2
                for h in (0, 1):
                    for p in range(npairs):
                        attn_pair(g, h, p, o_ps[h])
                    def norm(o_ps=o_ps[h], h=h, onorm16=onorm16, g=g):
                        flush_pv(all=True)  # o must be complete before read
                        if DEBUG and (g, h) == DBG_GHP[:2]:
                            dof = w.tile([128, 4 * 65], F32, tag="dbgo")
                            nc.vector.tensor_copy(
                                dof.rearrange("p (a b) -> p a b", a=4),
                                o_ps[:, :, 0:65])
                            nc.sync.dma_start(
                                out=dbg_o.rearrange("p a b -> p (a b)"),
                                in_=dof)
                        rec_s = sb.tile([128, 4], F32, tag="rec")
                        with nc.allow_low_precision(reason="softmax sum"):
                            nc.vector.reciprocal(
                                rec_s,
                                o_ps[:, :, 64:65].rearrange("p a b -> p (a b)"))
                        nc.vector.tensor_tensor(
                            onorm16[:, :, h, :], o_ps[:, :, 0:64],
                            rec_s.unsqueeze(2).broadcast_to([128, 4, 64]),
                            mybir.AluOpType.mult,
                        )
                    deferred.append(norm)

            def attn_pair(g, h, p, o_ps):
                if h == 0 and p == 2 * g:
                    while proj_pending and proj_pending[0][0] <= g:
                        proj_pending.pop(0)[2]()
                hb = h * 64
                js = (2 * p, 2 * p + 1)
                sc_ps = psA.tile([128, 1024], F32, tag="bigA")
                offs = [_toff(j - 4 * g) for j in js]
                starts = [offs[0], 512]
                ends = [starts[i] + 512 - offs[i] for i in range(2)]
                for idx, j in enumerate(js):
                    nc.tensor.matmul(
                        sc_ps[:, starts[idx]:ends[idx]],
                        kt_s[hb:hb + 64, bass.ts(j, 128)],
                        qt_s[hb:hb + 64, g * 512 + offs[idx]:(g + 1) * 512],
                        start=True, stop=True,
                    )
                at_s = sbA.tile([128, 1024], BF16, tag="attn")
                offl = p < 2 * g and p % SCH_MOD == 0
                if offl:
                    i1_s = sbI.tile([128, 1024], I16, tag="i1")
                    nc.vector.tensor_scalar(
                        i1_s, sc_ps, SCH_A, SCH_B1,
                        mybir.AluOpType.mult, mybir.AluOpType.add,
                    )
                    i2_s = sbI.tile([128, 1024], I16, tag="i2")
                    nc.vector.tensor_scalar(
                        i2_s, i1_s, 64.0, None, mybir.AluOpType.add)
                else:
                    nc.scalar.activation(
                        at_s[:, starts[0]:ends[-1]],
                        sc_ps[:, starts[0]:ends[-1]],
                        mybir.ActivationFunctionType.Exp,
                    )
                    if DEBUG and (g, h, p) == DBG_GHP:
                        datf = w.tile([128, 1024], F32, tag="dbgat")
                        nc.vector.memset(datf, 0.0)
                        nc.vector.tensor_copy(
                            datf[:, starts[0]:ends[-1]],
                            at_s[:, starts[0]:ends[-1]])
                        nc.sync.dma_start(out=dbg_at[:], in_=datf)
                    if p >= 2 * g:
                        # causal triangles, post-exp 0/1 multiply on Pool
                        if p == 2 * g:
                            v2 = at_s.rearrange(
                                "p (a b) -> p a b", a=2)[:, :, 0:128]
                        else:
                            v2 = at_s[:, 256:1024].rearrange(
                                "p (a b) -> p a b", a=2)[:, :, 0:128]
                        nc.gpsimd.tensor_tensor(
                            v2, v2,
                            tri01_s.unsqueeze(1).broadcast_to([128, 2, 128]),
                            mybir.AluOpType.mult,
                        )
                flush_pv()
                if proj_pending:
                    proj_pending.pop(0)[2]()
                if deferred:
                    deferred.pop(0)()

                if offl:
                    ats = ((i1_s.bitcast(BF16), v16_s),
                           (i2_s.bitcast(BF16), v16c_s))
                else:
                    ats = ((at_s, v16_s),)

                def pv(js=js, offs=offs, starts=starts,
                       ats=ats, o_ps=o_ps, h=h, g=g):
                    for idx, j in enumerate(js):
                        d = j - 4 * g
                        to = offs[idx]
                        for qt in range(4):
                            if d > qt:
                                continue
                            col = starts[idx] + qt * 128 - to
                            for ai, (a_t, v_t) in enumerate(ats):
                                # start=True zeroes the whole PSUM bank:
                                # only the first write to the bank gets it
                                nc.tensor.matmul(
                                    o_ps[:, qt, 0:65],
                                    a_t[:, col:col + 128],
                                    v_t[:, j, h, :],
                                    start=(j == 0 and qt == 0 and ai == 0),
                                    stop=(j == 4 * g + qt
                                          and ai == len(ats) - 1),
                                )
                pv_pending.append(pv)

            # ---- startup: weights + first two column groups ----
            nc.sync.dma_start(out=wq_s, in_=wq.rearrange("c p m -> p c m"))
            nc.sync.dma_start(out=sblob_s, in_=sblob[:])
            # touch Exp once so the ACT table loads during the startup DMAs
            warm_s = sb.tile([1, 1], F32, tag="warm")
            nc.scalar.activation(warm_s, qb_s[0:1, 0:1],
                                 mybir.ActivationFunctionType.Exp)
            for ch in range(4):
                eng = nc.sync if ch % 2 == 0 else nc.gpsimd
                eng.dma_start(out=xt_s[:, ch, bass.ts(0, 512)],
                              in_=xt[ch][:, bass.ts(0, 512)])
            nc.sync.dma_start(out=wk_s, in_=wk.rearrange("c p m -> p c m"))
            nc.sync.dma_start(out=wvt_s, in_=wvt.rearrange("c p m -> p c m"))
            # softmax row-sum ones-columns of V_aug
            nc.vector.memset(v16_s[:, :, :, 64:65], 1.0)
            nc.vector.memset(c2_s, SCH_C2)
            proj(0, skip_dma=True)
            nc.sync.dma_start(out=wout_s, in_=wout.rearrange("p c m -> p c m"))

            for g in range(8):
                if g < 7:
                    queue_proj(g + 1)
                onorm16 = sb.tile([128, 4, 2, 64], BF16, tag="onorm")
                onT = sb.tile([128, 512], BF16, tag="onT")
                attn_g(g, onorm16)

                for qt in range(4):
                    def tr(g=g, qt=qt, onorm16=onorm16, onT=onT):
                        trans_o(g, qt, onorm16, onT)
                    deferred.append(tr)
                if DEBUG and g == DBG_GHP[0]:
                    def dumpon(onT=onT):
                        donf = w.tile([128, 512], F32, tag="dbgon")
                        nc.vector.tensor_copy(donf, onT)
                        nc.sync.dma_start(out=dbg_on[:], in_=donf)
                    deferred.append(dumpon)
                for m in range(4):
                    def op(g=g, onT=onT, m=m):
                        outproj_m(g, onT, m, tail=(g == 7))
                    deferred.append(op)
            flush_pv(all=True)
            for fn in deferred:
                fn()
            if DEBUG:
                nc.sync.dma_start(out=dbg_q[:], in_=qt_s.bitcast(F32))
                nc.sync.dma_start(out=dbg_k[:], in_=kt_s.bitcast(F32))
                dvf = w.tile([128, 32 * 2 * 65], F32, tag="dbgv")
                nc.vector.tensor_copy(
                    dvf.rearrange("p (a b c) -> p a b c", a=32, b=2), v16_s)
                nc.sync.dma_start(
                    out=dbg_v.rearrange("p a b c -> p (a b c)"), in_=dvf)
    nc.compile()
    return nc


def _pack_inputs(x, Wqkv, bqkv, Wout, bout):
    """Per-core input dicts."""
    bf16 = mybir.dt.np(BF16)
    idx = np.arange(128)
    tri01 = np.ascontiguousarray(
        np.where(idx[None, :] >= idx[:, None], 1.0, 0.0).astype(bf16)
    ).view(np.float32)
    ident16 = np.ascontiguousarray(np.eye(128, dtype=bf16)).view(np.float32)
    in_maps = []
    for c in range(NCORES):
        b = c // 4
        h0 = 2 * (c % 4)
        xt = np.ascontiguousarray(x[b].T).reshape(4, 128, T)
        wq = np.ascontiguousarray(
            Wqkv[:, h0 * 64:h0 * 64 + 128].reshape(4, 128, 128))
        wk = np.ascontiguousarray(
            Wqkv[:, 512 + h0 * 64:512 + h0 * 64 + 128].reshape(4, 128, 128))
        wvt = np.ascontiguousarray(
            Wqkv[:, 1024 + h0 * 64:1024 + h0 * 64 + 128].reshape(4, 128, 128))
        sblob = np.zeros((128, 135), dtype=np.float32)
        sblob[:, 0:1] = (bqkv[h0 * 64:h0 * 64 + 128] * SCALE
                         ).reshape(128, 1).astype(np.float32)
        sblob[:, 1:2] = bqkv[512 + h0 * 64:512 + h0 * 64 + 128
                             ].reshape(128, 1).astype(np.float32)
        sblob[:, 2:3] = bqkv[1024 + h0 * 64:1024 + h0 * 64 + 128
                             ].reshape(128, 1).astype(np.float32)
        if c % 4 == 0:
            sblob[:, 3:7] = np.ascontiguousarray(bout.reshape(4, 128).T)
        sblob[:, 7:71] = ident16
        sblob[:, 71:135] = tri01
        wout_c = np.ascontiguousarray(
            Wout[h0 * 64:h0 * 64 + 128, :].reshape(128, 4, 128)).astype(bf16)
        in_maps.append({
            "xt": np.ascontiguousarray(xt, dtype=np.float32),
            "wq": wq.astype(np.float32), "wk": wk.astype(np.float32),
            "wvt": wvt.astype(np.float32),
            "wout": wout_c,
            "sblob": sblob.copy(),
        })
    return in_maps


def kernel(x, Wqkv, bqkv, Wout, bout):
    global _NC, LAST_RESULT
    x = np.asarray(x, dtype=np.float32)
    Wqkv = np.asarray(Wqkv, dtype=np.float32)
    bqkv = np.asarray(bqkv, dtype=np.float32)
    Wout = np.asarray(Wout, dtype=np.float32)
    bout = np.asarray(bout, dtype=np.float32)

    if _NC is None:
        _NC = _build(bias_free=not (np.any(bqkv) or np.any(bout)))
    in_maps = _pack_inputs(x, Wqkv, bqkv, Wout, bout)
    res = run_bass_kernel_spmd(_NC, in_maps, list(range(NCORES)), trace=TRACE)
    LAST_RESULT = res
    out = np.zeros((B, T, C), dtype=np.float32)
    for c in range(NCORES):
        out[c // 4] += res.results[c]["out_t"].T
    return out


# revision 42
# speedup vs baseline: 1.0935x; 1.0935x over previous
"""Multi-head causal self-attention (B=2, T=4096, C=512, H=8) on 8 trn2 cores.

Sharding: 16 (batch, head) pairs -> 2 heads per core. Core c handles batch
c//4, heads {2*(c%4), 2*(c%4)+1}. Each core computes its heads' Q/K/V
projections from the (host-pre-transposed) activations, runs causal flash
attention, and applies its row-slice of the output projection; the host sums
the 4 partial outputs per batch.

Attention layout: scores are computed transposed ([tk, tq]) in fp32r into
PSUM; softmax weights are bf16. PV runs in [tq, d] layout (stationary =
attention tile window, moving = V[k,d]+ones), which packs the full 128
output partitions per pass and makes the softmax row-sum a per-partition
scalar: normalization is one reciprocal + broadcast-multiply. Causal
triangle masks are 0/1 multiplies on the bf16 weights, run post-exp on the
otherwise-idle Pool engine (PV trails by 2 pairs, so they are off the
critical path). The normalized output is PE-transposed back to [d, tq]
(bf16) for the output projection.

The ACT engine's exp over every causal score element (~17.6M elems/core at
128 lanes) is the throughput floor, so 1/3 of the non-diagonal score pairs
compute softmax weights on DVE instead, via a dual Schraudolph bitcast exp:
i_k = round(A*s + B_k) as int16 (B2 = B1+64), exp(s) ~= bf16_bits(i1) +
2^-0.5 * bf16_bits(i2). The two terms are combined by the PE itself (PV
runs twice, the second pass against a 2^-0.5-prescaled V copy), so the
offload costs DVE only one PSUM read (i1) plus one cheap int16 add. Max
weight error ~1%, which vanishes below bf16 noise after softmax
normalization (verified end-to-end).
"""

import numpy as np

import concourse.bass as bass
import concourse.mybir as mybir
import concourse.tile as tile
from concourse import bacc
from concourse.bass_utils import run_bass_kernel_spmd

B, T, C, H, D = 2, 4096, 512, 8, 64
NCORES = 8
SCALE = 1.0 / np.sqrt(D)

F32 = mybir.dt.float32
F32R = mybir.dt.float32r
BF16 = mybir.dt.bfloat16
I16 = mybir.dt.int16

TRACE = False
LAST_RESULT = None
# dual-Schraudolph exp offload (DVE/Pool) for non-diagonal score pairs:
# exp(x) ~= bf16_bits(round(A*x+B0-128)) + 2^-0.5 * bf16_bits(round(A*x+B0-64))
# (+-1.0% per weight, self-normalizing; end-to-end impact is below bf16 noise)
SCH_A = 128.0 / np.log(2.0)
SCH_B0 = 16249.125
SCH_B1 = SCH_B0 - 128.0
SCH_B2 = SCH_B0 - 64.0
SCH_C2 = 2.0 ** -0.5
SCH_MOD = 3
DEBUG = False  # adds intermediate dumps (dbg_*) for core-0 verification
DBG_GHP = (2, 0, 3)  # g, h, pair for the at_s dump

_NC = None


def _toff(d):
    """Column offset below which a diagonal block's scores are entirely
    invalid *and* skippable while keeping matmul N >= 256 (fp32r full rate)."""
    if d <= 0:
        return 0
    return 128 if d == 1 else 256


def _build(bias_free=False):
    nc = bacc.Bacc()

    xt = nc.declare_dram_parameter("xt", [4, 128, T], F32R, isOutput=False)
    wq = nc.declare_dram_parameter("wq", [4, 128, 128], F32R, isOutput=False)
    wk = nc.declare_dram_parameter("wk", [4, 128, 128], F32R, isOutput=False)
    wvt = nc.declare_dram_parameter("wvt", [4, 128, 128], F32R, isOutput=False)
    wout = nc.declare_dram_parameter("wout", [128, 4, 128], BF16, isOutput=False)
    # packed small constants: qb|kb|vbp|bout4|tri|ident16
    sblob = nc.declare_dram_parameter("sblob", [128, 135], F32, isOutput=False)
    out_t = nc.declare_dram_parameter("out_t", [C, T], F32, isOutput=True)
    if DEBUG:
        dbg_q = nc.declare_dram_parameter("dbg_q", [128, T], F32, isOutput=True)
        dbg_k = nc.declare_dram_parameter("dbg_k", [128, T], F32, isOutput=True)
        dbg_v = nc.declare_dram_parameter("dbg_v", [128, 32, 2, 65], F32,
                                          isOutput=True)
        dbg_at = nc.declare_dram_parameter("dbg_at", [128, 1024], F32,
                                           isOutput=True)
        dbg_o = nc.declare_dram_parameter("dbg_o", [128, 4, 65], F32,
                                          isOutput=True)
        dbg_on = nc.declare_dram_parameter("dbg_on", [128, 512], F32,
                                           isOutput=True)

    with tile.TileContext(nc) as tc:
        with (
            tc.tile_pool(name="w", bufs=1) as w,
            tc.tile_pool(name="sb", bufs=4) as sb,
            tc.tile_pool(name="sbA", bufs=11) as sbA,
            tc.tile_pool(name="sbI", bufs=9) as sbI,
            tc.tile_pool(name="psA", bufs=2, space="PSUM") as psA,
            tc.tile_pool(name="psO", bufs=2, space="PSUM") as psO,
            tc.tile_pool(name="psX", bufs=2, space="PSUM") as psX,
        ):
            # ---- weights / constants ----
            wq_s = w.tile([128, 4, 128], F32R)
            wk_s = w.tile([128, 4, 128], F32R)
            wvt_s = w.tile([128, 4, 128], F32R)
            wout_s = w.tile([128, 4, 128], BF16)
            sblob_s = w.tile([128, 135], F32)
            qb_s = sblob_s[:, 0:1]
            kb_s = sblob_s[:, 1:2]
            vbp_s = sblob_s[:, 2:3]
            bout_s = sblob_s[:, 3:7]
            ident_s = sblob_s[:, 7:71].bitcast(BF16)   # [128,128] bf16
            tri01_s = sblob_s[:, 71:135].bitcast(BF16)  # causal 0/1, bf16

            xt_s = w.tile([128, 4, T], F32R)
            qt_s = w.tile([128, T], F32R)  # partitions: [h0 q-dims | h1 q-dims]
            kt_s = w.tile([128, T], F32R)
            vt_s = w.tile([128, T], BF16)  # V^T stream: partitions [h0 d|h1 d]
            # per k-tile: [2 heads, 64 d + 1 ones]
            v16_s = w.tile([128, 32, 2, 65], BF16)
            v16c_s = w.tile([128, 32, 2, 65], BF16)  # 2^-0.5 * v16 (dual-schr)
            c2_s = w.tile([128, 1], BF16)


            def _proj_half(g, ws, dst, scale, bias, half, state, dt):
                sl = bass.ts(g, 512)
                if half == 0:
                    pproj = psX.tile([128, 512], F32, tag="x")
                    state["ps"] = pproj
                ps = state["ps"]
                for ch in (0, 1) if half == 0 else (2, 3):
                    nc.tensor.matmul(
                        ps, ws[:, ch, :], xt_s[:, ch, sl],
                        start=(ch == 0), stop=(ch == 3),
                    )
                if half == 1:
                    nc.vector.tensor_scalar(
                        dst[:, sl], ps, scale, bias,
                        mybir.AluOpType.mult, mybir.AluOpType.add,
                    )
                    state.pop("ps")

            def proj_q(g, half=None, state={}):
                for hf in (0, 1) if half is None else (half,):
                    _proj_half(g, wq_s, qt_s, SCALE, qb_s, hf, state, F32R)

            def proj_k(g, half=None, state={}):
                for hf in (0, 1) if half is None else (half,):
                    _proj_half(g, wk_s, kt_s, 1.0, kb_s, hf, state, F32R)

            def proj_vt(g, half=None, state={}):
                for hf in (0, 1) if half is None else (half,):
                    _proj_half(g, wvt_s, vt_s, 1.0, vbp_s, hf, state, BF16)

            def trans_v(g, t4):
                tt = g * 4 + t4
                pt = psX.tile([128, 512], F32, tag="x")
                ptb = pt.bitcast(BF16)
                nc.tensor.transpose(
                    ptb[:, 0:128], vt_s[:, bass.ts(tt, 128)], ident_s,
                )
                nc.vector.tensor_copy(
                    v16_s[:, tt, :, 0:64],
                    ptb[:, 0:128].rearrange("p (a b) -> p a b", a=2),
                )
                nc.gpsimd.tensor_tensor(
                    v16c_s[:, tt, :, :].rearrange("p a b -> p (a b)"),
                    v16_s[:, tt, :, :].rearrange("p a b -> p (a b)"),
                    c2_s.broadcast_to([128, 130]),
                    mybir.AluOpType.mult,
                )

            def proj(g, skip_dma=False):
                """QKV projection for column group g, emitted inline."""
                if not skip_dma:
                    sl = bass.ts(g, 512)
                    for ch in range(4):
                        nc.sync.dma_start(out=xt_s[:, ch, sl], in_=xt[ch][:, sl])
                proj_q(g)
                proj_k(g)
                proj_vt(g)
                for t4 in range(4):
                    trans_v(g, t4)

            def queue_proj(g):
                """Queue proj(g) pieces for drip-feeding under attention."""
                sl = bass.ts(g, 512)
                for ch in range(4):
                    nc.sync.dma_start(out=xt_s[:, ch, sl], in_=xt[ch][:, sl])
                for late, fn in ((0, proj_q), (1, proj_k), (1, proj_vt)):
                    st = {}
                    for hf in (0, 1):
                        proj_pending.append(
                            (g, late,
                             lambda g=g, fn=fn, hf=hf, st=st: fn(g, hf, st)))
                for t4 in range(4):
                    proj_pending.append(
                        (g, 1, lambda g=g, t4=t4: trans_v(g, t4)))

            def trans_o(g, qt, onorm16, onT):
                pt = psX.tile([128, 512], F32, tag="x")
                ptb = pt.bitcast(BF16)
                nc.tensor.transpose(
                    ptb[:, 0:128],
                    onorm16[:, qt].rearrange("p a b -> p (a b)"), ident_s,
                )
                nc.vector.tensor_copy(
                    onT[:, bass.ts(qt, 128)], ptb[:, 0:128])

            def outproj_m(g, onT, m, tail=False):
                """One column-chunk of the output projection for q-chunk g
                (deferred so it fills PE gaps under later attention)."""
                if tail:
                    op_full = psA.tile([128, 1024], F32, tag="bigA")
                    op_ps = op_full[:, 0:512]
                else:
                    op_ps = psX.tile([128, 512], F32, tag="x")
                nc.tensor.matmul(
                    op_ps, wout_s[:, m, :], onT,
                    start=True, stop=True,
                )
                oc_s = sb.tile([128, 512], F32, tag="outc")
                nc.vector.tensor_scalar(
                    oc_s, op_ps, 1.0, bout_s[:, m:m + 1],
                    mybir.AluOpType.mult, mybir.AluOpType.add,
                )
                nc.sync.dma_start(
                    out=out_t[bass.ts(m, 128), bass.ts(g, 512)], in_=oc_s
                )

            pv_pending = []
            deferred = []
            proj_pending = []

            def flush_pv(all=False):
                # keep up to 2 pending pv closures so PV matmuls only enter
                # the PE queue after their exp has certainly completed
                while pv_pending and (all or len(pv_pending) > 6):
                    pv_pending.pop(0)()

            def attn_g(g, onorm16):
                """Both heads' causal attention over q-chunk g, pair chains
                interleaved so the psA/at rotation always has an independent
                chain to run while the other waits on exp/conversions."""
                while proj_pending and (
                    proj_pending[0][0] < g
                    or (proj_pending[0][0] == g and proj_pending[0][1] == 0)
                ):
                    proj_pending.pop(0)[2]()
                o_ps = {h: psO.tile([128, 4, 128], F32, tag="o",
                                    name=f"o_ps{h}")
                        for h in (0, 1)}
                npairs = 2 * g + 2
                for h in (0, 1):
                    for p in range(npairs):
                        attn_pair(g, h, p, o_ps[h])
                    def norm(o_ps=o_ps[h], h=h, onorm16=onorm16, g=g):
                        flush_pv(all=True)  # o must be complete before read
                        if DEBUG and (g, h) == DBG_GHP[:2]:
                            dof = w.tile([128, 4 * 65], F32, tag="dbgo")
                            nc.vector.tensor_copy(
                                dof.rearrange("p (a b) -> p a b", a=4),
                                o_ps[:, :, 0:65])
                            nc.sync.dma_start(
                                out=dbg_o.rearrange("p a b -> p (a b)"),
                                in_=dof)
                        rec_s = sb.tile([128, 4], F32, tag="rec")
                        with nc.allow_low_precision(reason="softmax sum"):
                            nc.vector.reciprocal(
                                rec_s,
                                o_ps[:, :, 64:65].rearrange("p a b -> p (a b)"))
                        nc.vector.tensor_tensor(
                            onorm16[:, :, h, :], o_ps[:, :, 0:64],
                            rec_s.unsqueeze(2).broadcast_to([128, 4, 64]),
                            mybir.AluOpType.mult,
                        )
                    deferred.append(norm)

            def attn_pair(g, h, p, o_ps):
                if h == 0 and p == 2 * g:
                    while proj_pending and proj_pending[0][0] <= g:
                        proj_pending.pop(0)[2]()
                hb = h * 64
                js = (2 * p, 2 * p + 1)
                sc_ps = psA.tile([128, 1024], F32, tag="bigA")
                offs = [_toff(j - 4 * g) for j in js]
                starts = [offs[0], 512]
                ends = [starts[i] + 512 - offs[i] for i in range(2)]
                for idx, j in enumerate(js):
                    nc.tensor.matmul(
                        sc_ps[:, starts[idx]:ends[idx]],
                        kt_s[hb:hb + 64, bass.ts(j, 128)],
                        qt_s[hb:hb + 64, g * 512 + offs[idx]:(g + 1) * 512],
                        start=True, stop=True,
                    )
                at_s = sbA.tile([128, 1024], BF16, tag="attn")
                offl = p < 2 * g and p % SCH_MOD == 0
                if offl:
                    i1_s = sbI.tile([128, 1024], I16, tag="i1")
                    nc.vector.tensor_scalar(
                        i1_s, sc_ps, SCH_A, SCH_B1,
                        mybir.AluOpType.mult, mybir.AluOpType.add,
                    )
                    i2_s = sbI.tile([128, 1024], I16, tag="i2")
                    nc.vector.tensor_scalar(
                        i2_s, i1_s, 64.0, None, mybir.AluOpType.add)
                else:
                    nc.scalar.activation(
                        at_s[:, starts[0]:ends[-1]],
                        sc_ps[:, starts[0]:ends[-1]],
                        mybir.ActivationFunctionType.Exp,
                    )
                    if DEBUG and (g, h, p) == DBG_GHP:
                        datf = w.tile([128, 1024], F32, tag="dbgat")
                        nc.vector.memset(datf, 0.0)
                        nc.vector.tensor_copy(
                            datf[:, starts[0]:ends[-1]],
                            at_s[:, starts[0]:ends[-1]])
                        nc.sync.dma_start(out=dbg_at[:], in_=datf)
                    if p >= 2 * g:
                        # causal triangles, post-exp 0/1 multiply on Pool
                        if p == 2 * g:
                            v2 = at_s.rearrange(
                                "p (a b) -> p a b", a=2)[:, :, 0:128]
                        else:
                            v2 = at_s[:, 256:1024].rearrange(
                                "p (a b) -> p a b", a=2)[:, :, 0:128]
                        nc.gpsimd.tensor_tensor(
                            v2, v2,
                            tri01_s.unsqueeze(1).broadcast_to([128, 2, 128]),
                            mybir.AluOpType.mult,
                        )
                flush_pv()
                if proj_pending:
                    proj_pending.pop(0)[2]()
                elif deferred:
                    deferred.pop(0)()

                if offl:
                    ats = ((i1_s.bitcast(BF16), v16_s),
                           (i2_s.bitcast(BF16), v16c_s))
                else:
                    ats = ((at_s, v16_s),)

                def pv(js=js, offs=offs, starts=starts,
                       ats=ats, o_ps=o_ps, h=h, g=g):
                    for idx, j in enumerate(js):
                        d = j - 4 * g
                        to = offs[idx]
                        for qt in range(4):
                            if d > qt:
                                continue
                            col = starts[idx] + qt * 128 - to
                            for ai, (a_t, v_t) in enumerate(ats):
                                # start=True zeroes the whole PSUM bank:
                                # only the first write to the bank gets it
                                nc.tensor.matmul(
                                    o_ps[:, qt, 0:65],
                                    a_t[:, col:col + 128],
                                    v_t[:, j, h, :],
                                    start=(j == 0 and qt == 0 and ai == 0),
                                    stop=(j == 4 * g + qt
                                          and ai == len(ats) - 1),
                                )
                pv_pending.append(pv)

            # ---- startup: weights + first two column groups ----
            nc.sync.dma_start(out=wq_s, in_=wq.rearrange("c p m -> p c m"))
            nc.sync.dma_start(out=sblob_s, in_=sblob[:])
            # touch Exp once so the ACT table loads during the startup DMAs
            warm_s = sb.tile([1, 1], F32, tag="warm")
            nc.scalar.activation(warm_s, qb_s[0:1, 0:1],
                                 mybir.ActivationFunctionType.Exp)
            for ch in range(4):
                eng = nc.sync if ch % 2 == 0 else nc.gpsimd
                eng.dma_start(out=xt_s[:, ch, bass.ts(0, 512)],
                              in_=xt[ch][:, bass.ts(0, 512)])
            nc.sync.dma_start(out=wk_s, in_=wk.rearrange("c p m -> p c m"))
            nc.sync.dma_start(out=wvt_s, in_=wvt.rearrange("c p m -> p c m"))
            # softmax row-sum ones-columns of V_aug
            nc.vector.memset(v16_s[:, :, :, 64:65], 1.0)
            nc.vector.memset(c2_s, SCH_C2)
            proj(0, skip_dma=True)
            nc.sync.dma_start(out=wout_s, in_=wout.rearrange("p c m -> p c m"))

            for g in range(8):
                if g < 7:
                    queue_proj(g + 1)
                onorm16 = sb.tile([128, 4, 2, 64], BF16, tag="onorm")
                onT = sb.tile([128, 512], BF16, tag="onT")
                attn_g(g, onorm16)

                for qt in range(4):
                    def tr(g=g, qt=qt, onorm16=onorm16, onT=onT):
                        trans_o(g, qt, onorm16, onT)
                    deferred.append(tr)
                if DEBUG and g == DBG_GHP[0]:
                    def dumpon(onT=onT):
                        donf = w.tile([128, 512], F32, tag="dbgon")
                        nc.vector.tensor_copy(donf, onT)
                        nc.sync.dma_start(out=dbg_on[:], in_=donf)
                    deferred.append(dumpon)
                for m in range(4):
                    def op(g=g, onT=onT, m=m):
                        outproj_m(g, onT, m, tail=(g == 7))
                    deferred.append(op)
            flush_pv(all=True)
            for fn in deferred:
                fn()
            if DEBUG:
                nc.sync.dma_start(out=dbg_q[:], in_=qt_s.bitcast(F32))
                nc.sync.dma_start(out=dbg_k[:], in_=kt_s.bitcast(F32))
                dvf = w.tile([128, 32 * 2 * 65], F32, tag="dbgv")
                nc.vector.tensor_copy(
                    dvf.rearrange("p (a b c) -> p a b c", a=32, b=2), v16_s)
                nc.sync.dma_start(
                    out=dbg_v.rearrange("p a b c -> p (a b c)"), in_=dvf)
    nc.compile()
    return nc


def _pack_inputs(x, Wqkv, bqkv, Wout, bout):
    """Per-core input dicts."""
    bf16 = mybir.dt.np(BF16)
    idx = np.arange(128)
    tri01 = np.ascontiguousarray(
        np.where(idx[None, :] >= idx[:, None], 1.0, 0.0).astype(bf16)
    ).view(np.float32)
    ident16 = np.ascontiguousarray(np.eye(128, dtype=bf16)).view(np.float32)
    in_maps = []
    for c in range(NCORES):
        b = c // 4
        h0 = 2 * (c % 4)
        xt = np.ascontiguousarray(x[b].T).reshape(4, 128, T)
        wq = np.ascontiguousarray(
            Wqkv[:, h0 * 64:h0 * 64 + 128].reshape(4, 128, 128))
        wk = np.ascontiguousarray(
            Wqkv[:, 512 + h0 * 64:512 + h0 * 64 + 128].reshape(4, 128, 128))
        wvt = np.ascontiguousarray(
            Wqkv[:, 1024 + h0 * 64:1024 + h0 * 64 + 128].reshape(4, 128, 128))
        sblob = np.zeros((128, 135), dtype=np.float32)
        sblob[:, 0:1] = (bqkv[h0 * 64:h0 * 64 + 128] * SCALE
                         ).reshape(128, 1).astype(np.float32)
        sblob[:, 1:2] = bqkv[512 + h0 * 64:512 + h0 * 64 + 128
                             ].reshape(128, 1).astype(np.float32)
        sblob[:, 2:3] = bqkv[1024 + h0 * 64:1024 + h0 * 64 + 128
                             ].reshape(128, 1).astype(np.float32)
        if c % 4 == 0:
            sblob[:, 3:7] = np.ascontiguousarray(bout.reshape(4, 128).T)
        sblob[:, 7:71] = ident16
        sblob[:, 71:135] = tri01
        wout_c = np.ascontiguousarray(
            Wout[h0 * 64:h0 * 64 + 128, :].reshape(128, 4, 128)).astype(bf16)
        in_maps.append({
            "xt": np.ascontiguousarray(xt, dtype=np.float32),
            "wq": wq.astype(np.float32), "wk": wk.astype(np.float32),
            "wvt": wvt.astype(np.float32),
            "wout": wout_c,
            "sblob": sblob.copy(),
        })
    return in_maps


def kernel(x, Wqkv, bqkv, Wout, bout):
    global _NC, LAST_RESULT
    x = np.asarray(x, dtype=np.float32)
    Wqkv = np.asarray(Wqkv, dtype=np.float32)
    bqkv = np.asarray(bqkv, dtype=np.float32)
    Wout = np.asarray(Wout, dtype=np.float32)
    bout = np.asarray(bout, dtype=np.float32)

    if _NC is None:
        _NC = _build(bias_free=not (np.any(bqkv) or np.any(bout)))
    in_maps = _pack_inputs(x, Wqkv, bqkv, Wout, bout)
    res = run_bass_kernel_spmd(_NC, in_maps, list(range(NCORES)), trace=TRACE)
    LAST_RESULT = res
    out = np.zeros((B, T, C), dtype=np.float32)
    for c in range(NCORES):
        out[c // 4] += res.results[c]["out_t"].T
    return out
